# revision 6
# baseline (speedup 1.0000x reference)
"""v4: softmax fully on host; device does only the PV merge (99.3% of FLOPs).

out[b] = w[b].T @ meg[b], w = softmax(emb @ heads.T) computed on host in
f32 BLAS (~60 ms), uploaded as packed fp16 [128, 3240] per core (1 DMA).
Device: pure PV stream, weights stationary amortized over 2048 cols.
"""

import math

import numpy as np

import concourse.bass as bass
import concourse.mybir as mybir
import concourse.tile as tile
from concourse import bacc

F32 = mybir.dt.float32
F16 = mybir.dt.float16

B, C, T = 32, 273, 8192
O, D = 270, 288
N_CORES = 8
BPC = B // N_CORES
MARGIN = 0.2
N_FREQ = 12
TWO_PI = 2.0 * math.pi

TS = 4096
NST = T // TS

C_CHUNKS = [(0, 128), (128, 128), (256, C - 256)]
O_CHUNKS = [(0, 128), (128, 128), (256, O - 256)]


def _build_module() -> bass.Bass:
    nc = bacc.Bacc()
    meg_h = nc.dram_tensor("meg", [BPC, C, T], F16, kind="ExternalInput")
    # packed softmax weights: 12 column-blocks of 270; block 3*b+ci =
    # w[b, c0:c0+csz, :] at rows 0:csz
    w_h = nc.dram_tensor("wp", [128, 3 * BPC * O], F16, kind="ExternalInput")
    out_h = nc.dram_tensor("out", [BPC, O, T], F16, kind="ExternalOutput")

    with tile.TileContext(nc) as tc:
        with (
            tc.tile_pool(name="const", bufs=1) as const,
            tc.tile_pool(name="megp", bufs=3) as megp,
            tc.tile_pool(name="outp", bufs=6) as outp,
            tc.tile_pool(name="psum", bufs=2, space="PSUM") as psum,
        ):
            ev = 0
            for b in range(BPC):
                wb = const.tile([128, 3 * O], F16, tag=f"w{b}", name=f"w{b}")
                nc.sync.dma_start(
                    out=wb, in_=w_h[:, 3 * b * O : 3 * (b + 1) * O]
                )
                for ts in range(NST):
                    t0 = ts * TS
                    megs = []
                    for ci, (c0, csz) in enumerate(C_CHUNKS):
                        m_ = megp.tile([csz, TS], F16, tag=f"meg{ci}", name=f"meg{ci}")
                        # h-split loads: PV on the first half starts ~2.5us
                        # earlier (slice-level tile deps)
                        for hh in range(2):
                            nc.sync.dma_start(
                                out=m_[:, hh * 2048 : (hh + 1) * 2048],
                                in_=meg_h[
                                    b,
                                    c0 : c0 + csz,
                                    t0 + hh * 2048 : t0 + (hh + 1) * 2048,
                                ],
                            )
                        megs.append(m_)
                    for oi, (o0, osz) in enumerate(O_CHUNKS):
                        ostage = outp.tile([128, TS], F16, tag="ostage", name="ostage")[
                            :osz
                        ]
                        for h in range(TS // 2048):
                            pv_ps = psum.tile(
                                [128, 2048], F32, tag="ps", name=f"pv{h}"
                            )[:osz]
                            h0 = h * 2048
                            for ci, (c0, csz) in enumerate(C_CHUNKS):
                                w_ = wb[:csz, ci * O + o0 : ci * O + o0 + osz]
                                for sl in range(4):
                                    nc.tensor.matmul(
                                        pv_ps[:, sl * 512 : (sl + 1) * 512],
                                        w_,
                                        megs[ci][
                                            :, h0 + sl * 512 : h0 + (sl + 1) * 512
                                        ],
                                        start=(ci == 0),
                                        stop=(ci == 2),
                                    )
                            if ev % 2 == 0:
                                nc.vector.tensor_scalar_mul(
                                    ostage[:, h0 : h0 + 2048], pv_ps, 1.0
                                )
                            else:
                                nc.scalar.copy(ostage[:, h0 : h0 + 2048], pv_ps)
                            ev += 1
                            # h-split store: drain output as soon as the
                            # half-eviction lands (shrinks the tail)
                            nc.scalar.dma_start(
                                out=out_h[
                                    b, o0 : o0 + osz, t0 + h0 : t0 + h0 + 2048
                                ],
                                in_=ostage[:, h0 : h0 + 2048],
                            )
    nc.compile()
    return nc


_MODULE_CACHE: list = []


def _get_module() -> bass.Bass:
    if not _MODULE_CACHE:
        _MODULE_CACHE.append(_build_module())
    return _MODULE_CACHE[0]


def _host_softmax_w(positions, heads):
    """w[b, c, o] = softmax_c(emb @ heads.T) in f32."""
    freqs = (TWO_PI / (1.0 + 2.0 * MARGIN)) * np.arange(N_FREQ, dtype=np.float64)
    pos = positions.astype(np.float64) + MARGIN
    loc = (
        pos[..., 0][..., None, None] * freqs[:, None]
        + pos[..., 1][..., None, None] * freqs[None, :]
    ).reshape(B, C, N_FREQ * N_FREQ)
    emb = np.concatenate([np.cos(loc), np.sin(loc)], axis=2).astype(np.float32)
    scores = emb @ heads.T.astype(np.float32)            # [B, C, O]
    scores -= scores.max(axis=1, keepdims=True)
    np.exp(scores, out=scores)
    scores /= scores.sum(axis=1, keepdims=True)
    return scores                                         # [B, C, O] f32


def _host_prep(meg, positions, heads):
    w = _host_softmax_w(positions, heads)
    in_maps = []
    for core in range(N_CORES):
        wp = np.zeros((128, 3 * BPC * O), dtype=np.float16)
        for b in range(BPC):
            gb = core * BPC + b
            for ci, (c0, csz) in enumerate(C_CHUNKS):
                blk = (3 * b + ci) * O
                wp[:csz, blk : blk + O] = w[gb, c0 : c0 + csz]
        sl = slice(core * BPC, (core + 1) * BPC)
        in_maps.append(
            {
                "meg": np.ascontiguousarray(meg[sl]).astype(np.float16),
                "wp": wp,
            }
        )
    return in_maps


LAST_RESULTS = None


def kernel(meg: np.ndarray, positions: np.ndarray, heads: np.ndarray) -> np.ndarray:
    global LAST_RESULTS
    from concourse.bass_utils import run_bass_kernel_spmd

    nc = _get_module()
    in_maps = _host_prep(
        np.asarray(meg, dtype=np.float32),
        np.asarray(positions, dtype=np.float32),
        np.asarray(heads, dtype=np.float32),
    )
    res = run_bass_kernel_spmd(nc, in_maps, core_ids=list(range(N_CORES)))
    LAST_RESULTS = res
    return np.concatenate(
        [r["out"].astype(np.float32) for r in res.results], axis=0
    )
